# revision 26
# baseline (speedup 1.0000x reference)
"""LIF neuron scan kernel v4 for Trainium2 (8 NeuronCores).

Reference recurrence per timestep t (elementwise over B x N):
    u = (x_t - m)*sig + m ; s = (u >= th) ; m = (1-s)*u
with sig = sigmoid(tau_x) in (0,1), th > 0 per neuron.

uh-space (v3) reformulation kept: uh = u/th, xs = x*(sig/th), cm = 1-sig:
    w  = (uh < 1) * cm        # stt, fp32 (w==0 iff spike; w carries the decay)
    p  = uh * w               # tt fp32
    uh'= p + xs'              # tt fp32
State stays fp32 end-to-end: fp16-state variants flip ~2-3k near-threshold
spikes (emulated: 2.2-2.6e-2 rel err > the 2e-2 gate), so 16-bit state is
not usable; only non-state paths are 16-bit.

v4 changes vs v3 (probe-informed; see probe.py):
  - Input pipeline: one 2 MB dma_start per 8-t block, 3-slot ring with a
    per-slot semaphore (no cross-block issue throttle). v3 serialized input
    DMAs behind output waits + a completion throttle and DVE idled 4.8 us
    per block waiting for x.
  - mems path: ACT casts the p block to fp16 (ACT is ~70% idle), DVE does
    mems16 = p16 * thcm16 as an all-fp16 tensor_tensor -> 2x_1P mode
    (2.2 us vs 4.3 us per block). mems output fp16 (was bf16): same bytes,
    ~8x better mantissa. Accuracy impact ~5e-4 rel, well inside the gate.
  - spikes: one ACT Exp per block ([128,4096], fp8e4 out, exact 0/1),
    halving spike DMA bytes and cutting per-t semaphore traffic.
  - Chain ops carry no per-t then_inc except the block-final ones: pure TT
    pitch is 602 ns; v3's per-op sem traffic ran it at 722 ns.
  - GpSimd compute offload was probed and rejected: a gpsimd tensor_tensor
    blocks concurrent DVE almost completely (one DVE op per gpsimd op).
    SWDGE DMA traffic does NOT block DVE (probed) but is not needed.
  - DMA accum_op=mult (CCE) is rejected by the compiler ("DMACopy does not
    support mult with Copy mode") - multiply-during-DMA is not available.

Sharding: data-parallel over batch B across 8 cores (4 batches/core),
constants replicated; cores fully independent (recurrence is only over T).
Per-core layout: [128, 512] per timestep, partition p = b_local*32 + sub,
free = n_low, neuron n = sub*512 + n_low.
"""

import sys

if "/opt/trn_rl_repo" not in sys.path:
    sys.path.insert(0, "/opt/trn_rl_repo")

import contextlib

import numpy as np

import concourse.bass as bass
import concourse.mybir as mybir
from concourse.bass_utils import run_bass_kernel_spmd

B, T, N = 32, 64, 16384
NCORES = 8
BL = B // NCORES
SUB = 32
NL = N // SUB  # 512
P = BL * SUB  # 128
TBLK = 8
NBLK = T // TBLK
BW = TBLK * NL  # 4096
F32 = mybir.dt.float32
F16 = mybir.dt.float16
F8 = mybir.dt.float8e4
ALU = mybir.AluOpType
AF = mybir.ActivationFunctionType

_CACHE: dict = {}


def _build_nc() -> bass.Bass:
    nc = bass.Bass()
    x = nc.dram_tensor("x", [BL, T, N], F32, kind="ExternalInput")
    # host-pretiled consts: [128, 512], partition p=(b,s) -> neuron s*512+n
    cm_d = nc.dram_tensor("cm", [P, NL], F32, kind="ExternalInput")
    sigth_d = nc.dram_tensor("sigth", [P, NL], F32, kind="ExternalInput")
    thcm_d = nc.dram_tensor("thcm16", [P, NL], F16, kind="ExternalInput")
    spikes8 = nc.dram_tensor("spikes8", [BL, T, N], F8, kind="ExternalOutput")
    mems16 = nc.dram_tensor("mems16", [BL, T, N], F16, kind="ExternalOutput")

    def x_src(b, k):
        return x[b, k * TBLK : (k + 1) * TBLK, :].rearrange(
            "t (s n) -> s t n", n=NL
        )

    def out_dst(dram, b, k):
        return dram[b, k * TBLK : (k + 1) * TBLK, :].rearrange(
            "t (s n) -> s t n", n=NL
        )

    def bv(tile, b):
        return tile[b * SUB : (b + 1) * SUB, :].rearrange(
            "p (t n) -> p t n", n=NL
        )

    with contextlib.ExitStack() as st:
        xb_all = st.enter_context(nc.sbuf_tensor([P, 3 * BW], F32))
        xs_t = st.enter_context(nc.sbuf_tensor([P, BW], F32))
        sigthb = st.enter_context(nc.sbuf_tensor([P, BW], F32))
        thcmb16 = st.enter_context(nc.sbuf_tensor([P, BW], F16))
        cm_t = st.enter_context(nc.sbuf_tensor([P, NL], F32))
        sigth_t = st.enter_context(nc.sbuf_tensor([P, NL], F32))
        thcm16_t = st.enter_context(nc.sbuf_tensor([P, NL], F16))
        uh_t = st.enter_context(nc.sbuf_tensor([P, 2 * NL], F32))
        w_all = st.enter_context(nc.sbuf_tensor([P, 2 * BW], F32))
        p_all = st.enter_context(nc.sbuf_tensor([P, 2 * BW], F32))
        p16_t = st.enter_context(nc.sbuf_tensor([P, BW], F16))
        m16_all = st.enter_context(nc.sbuf_tensor([P, 2 * BW], F16))
        s8_t = st.enter_context(nc.sbuf_tensor([P, BW], F8))
        c_sem = st.enter_context(nc.semaphore("c_sem"))
        rep_sem = st.enter_context(nc.semaphore("rep_sem"))
        xs0_sem = st.enter_context(nc.semaphore("xs0_sem"))
        xs1_sem = st.enter_context(nc.semaphore("xs1_sem"))
        xs2_sem = st.enter_context(nc.semaphore("xs2_sem"))
        xsd_sem = st.enter_context(nc.semaphore("xsd_sem"))
        w_sem = st.enter_context(nc.semaphore("w_sem"))
        pb_sem = st.enter_context(nc.semaphore("pb_sem"))
        p16_sem = st.enter_context(nc.semaphore("p16_sem"))
        spk_sem = st.enter_context(nc.semaphore("spk_sem"))
        m16d_sem = st.enter_context(nc.semaphore("m16d_sem"))
        mo_sem = st.enter_context(nc.semaphore("mo_sem"))
        so_sem = st.enter_context(nc.semaphore("so_sem"))
        block = st.enter_context(nc.Block())

        xslot_sems = [xs0_sem, xs1_sem, xs2_sem]

        def xb_r(k):
            return xb_all[:, (k % 3) * BW : (k % 3 + 1) * BW]

        def wsl(k, tl):
            r = k % 2
            return w_all[:, (r * TBLK + tl) * NL : (r * TBLK + tl + 1) * NL]

        def wblk(k):
            r = k % 2
            return w_all[:, r * BW : (r + 1) * BW]

        def psl(k, tl):
            r = k % 2
            return p_all[:, (r * TBLK + tl) * NL : (r * TBLK + tl + 1) * NL]

        def pblk(k):
            r = k % 2
            return p_all[:, r * BW : (r + 1) * BW]

        def uhsl(t):
            r = t % 2
            return uh_t[:, r * NL : (r + 1) * NL]

        def m16sl(j):
            r = j % 2
            return m16_all[:, r * BW : (r + 1) * BW]

        @block.sync
        def _(sync):
            # consts first (tiny, unblock ACT replication), then x0 (b=0,1;
            # the other half is issued from the ACT queue), then x1/x2
            for src, dst in (
                (sigth_d, sigth_t),
                (cm_d, cm_t),
                (thcm_d, thcm16_t),
            ):
                sync.dma_start(out=dst[:, :], in_=src[:, :]).then_inc(c_sem, 16)
            for b in (0, 1):
                sync.dma_start(out=bv(xb_r(0), b), in_=x_src(b, 0)).then_inc(
                    xslot_sems[0], 16
                )
            for k in (1, 2):
                for b in range(BL):
                    sync.dma_start(out=bv(xb_r(k), b), in_=x_src(b, k)).then_inc(
                        xslot_sems[k % 3], 16
                    )
            # mid-loop x input DMAs are issued from the ACT queue (the other
            # HWDGE ring) so they never serialize behind the output waits here
            for k in range(NBLK):
                # outputs of block k (spikes) and k-1 (mems)
                sync.wait_ge(spk_sem, k + 1)
                for b in range(BL):
                    sync.dma_start(
                        out=out_dst(spikes8, b, k), in_=bv(s8_t, b)
                    ).then_inc(so_sem, 16)
                if k >= 1:
                    sync.wait_ge(m16d_sem, k)
                    for b in range(BL):
                        sync.dma_start(
                            out=out_dst(mems16, b, k - 1),
                            in_=bv(m16sl(k - 1), b),
                        ).then_inc(mo_sem, 16)
            sync.wait_ge(m16d_sem, NBLK)
            for b in range(BL):
                sync.dma_start(
                    out=out_dst(mems16, b, NBLK - 1),
                    in_=bv(m16sl(NBLK - 1), b),
                ).then_inc(mo_sem, 16)
            sync.wait_ge(so_sem, 64 * NBLK)
            sync.wait_ge(mo_sem, 64 * NBLK)

        @block.vector
        def _(vector):
            vector.wait_ge(c_sem, 32)  # cm_t loaded
            vector.wait_ge(rep_sem, TBLK)  # sigthb tiled
            for k in range(NBLK):
                vector.wait_ge(xslot_sems[k % 3], 64 * (k // 3 + 1))
                if k >= 2:
                    # p ring slot k%2: ACT cast of block k-2 must be done
                    vector.wait_ge(p16_sem, k - 1)
                # xs for block k
                nc.vector.tensor_tensor(
                    out=xs_t[:, :], in0=xb_r(k), in1=sigthb[:, :], op=ALU.mult
                ).then_inc(xsd_sem, 1)
                if k >= 1:
                    # deferred add: uh_{8k} = p_{8k-1} + xs_{8k}
                    nc.vector.tensor_tensor(
                        out=uhsl(8 * k),
                        in0=psl(k - 1, TBLK - 1),
                        in1=xs_t[:, 0:NL],
                        op=ALU.add,
                    )
                if k >= 2:
                    vector.wait_ge(spk_sem, k - 1)  # w ring WAR vs ACT exp
                for tl in range(TBLK):
                    t = k * TBLK + tl
                    if k >= 1 and tl == 4:
                        # mems16 for block k-1, placed mid-chain so the ACT
                        # cast (done ~7us after chain k-1) is never waited on
                        vector.wait_ge(p16_sem, k)
                        if k >= 3:
                            vector.wait_ge(mo_sem, 64 * (k - 2))  # m16 WAR
                        nc.vector.tensor_tensor(
                            out=m16sl(k - 1),
                            in0=p16_t[:, :],
                            in1=thcmb16[:, :],
                            op=ALU.mult,
                        ).then_inc(m16d_sem, 1)
                    uh = xs_t[:, 0:NL] if t == 0 else uhsl(t)
                    ins_w = nc.vector.scalar_tensor_tensor(
                        out=wsl(k, tl),
                        in0=uh,
                        scalar=1.0,
                        in1=cm_t[:, :],
                        op0=ALU.is_lt,
                        op1=ALU.mult,
                    )
                    if tl == TBLK - 1:
                        ins_w.then_inc(w_sem, 1)
                    ins_p = nc.vector.tensor_tensor(
                        out=psl(k, tl), in0=uh, in1=wsl(k, tl), op=ALU.mult
                    )
                    if tl == TBLK - 1:
                        ins_p.then_inc(pb_sem, 1)
                    if k == NBLK - 1:
                        # last block: mems per-t straight from fp32 p (mixed
                        # dtype TT) so the final mems DMA starts immediately
                        if tl == 0:
                            vector.wait_ge(mo_sem, 64 * (NBLK - 2))
                        ins_m = nc.vector.tensor_tensor(
                            out=m16sl(k)[:, tl * NL : (tl + 1) * NL],
                            in0=psl(k, tl),
                            in1=thcmb16[:, tl * NL : (tl + 1) * NL],
                            op=ALU.mult,
                        )
                        if tl == TBLK - 1:
                            ins_m.then_inc(m16d_sem, 1)
                    if tl < TBLK - 1:
                        nc.vector.tensor_tensor(
                            out=uhsl(t + 1),
                            in0=psl(k, tl),
                            in1=xs_t[:, (tl + 1) * NL : (tl + 2) * NL],
                            op=ALU.add,
                        )

        @block.scalar
        def _(scalar):
            # other half of x block 0 (parallel issue with the sync queue)
            for b in (2, 3):
                nc.scalar.dma_start(
                    out=bv(xb_r(0), b), in_=x_src(b, 0)
                ).then_inc(xslot_sems[0], 16)
            scalar.wait_ge(c_sem, 16)  # sigth_t loaded
            for tl in range(TBLK):
                nc.scalar.copy(
                    out=sigthb[:, tl * NL : (tl + 1) * NL], in_=sigth_t[:, :]
                ).then_inc(rep_sem, 1)
            scalar.wait_ge(c_sem, 48)
            for tl in range(TBLK):
                nc.scalar.copy(
                    out=thcmb16[:, tl * NL : (tl + 1) * NL], in_=thcm16_t[:, :]
                ).then_inc(rep_sem, 1)
            for k in range(NBLK):
                # spikes block k first (gates DVE's w ring + sync's s8-out):
                # w==0 iff spike; exp(-1e30*w) = 1/0 exactly
                scalar.wait_ge(w_sem, k + 1)
                if k >= 1:
                    scalar.wait_ge(so_sem, 64 * k)  # s8 WAR
                nc.scalar.activation(
                    s8_t[:, :], wblk(k), AF.Exp, scale=-1e30
                ).then_inc(spk_sem, 1)
                # issue x input DMA for block k+3 (other HWDGE ring; gated
                # only on the xs-op that frees the ring slot)
                kf = k + 3
                if kf < NBLK:
                    scalar.wait_ge(xsd_sem, k + 1)
                    for b in range(BL):
                        nc.scalar.dma_start(
                            out=bv(xb_r(kf), b), in_=x_src(b, kf)
                        ).then_inc(xslot_sems[kf % 3], 16)
                # p block k -> fp16 (for the all-16-bit mems mult); skipped
                # for the last block (its mems come straight from fp32 p)
                if k < NBLK - 1:
                    scalar.wait_ge(pb_sem, k + 1)
                    if k >= 1:
                        scalar.wait_ge(m16d_sem, k)  # p16 used by m16-op k-1
                    nc.scalar.copy(out=p16_t[:, :], in_=pblk(k)).then_inc(
                        p16_sem, 1
                    )

    return nc


def _get_nc() -> bass.Bass:
    if "nc" not in _CACHE:
        _CACHE["nc"] = _build_nc()
    return _CACHE["nc"]


def kernel(x, thresh, tau_x, _trace: bool = False, _tmpdir: str | None = None):
    x = np.ascontiguousarray(np.asarray(x, dtype=np.float32))
    thresh = np.ascontiguousarray(np.asarray(thresh, dtype=np.float32))
    tau_x = np.ascontiguousarray(np.asarray(tau_x, dtype=np.float32))
    assert x.shape == (B, T, N)

    # O(N) host-side constants; all O(B*T*N) math happens on-device.
    sig = (1.0 / (1.0 + np.exp(-tau_x.astype(np.float64)))).astype(np.float32)
    cm = (np.float32(1.0) - sig).astype(np.float32)
    sigth = (sig / thresh).astype(np.float32)
    thcm16 = (thresh / cm).astype(np.float16)
    # pretile to [128, 512]: partition p = b_local*32 + s holds neuron
    # chunk s; replicate the [32, 512] view across the 4 b-groups
    cm_tl = np.ascontiguousarray(np.tile(cm.reshape(SUB, NL), (BL, 1)))
    sigth_tl = np.ascontiguousarray(np.tile(sigth.reshape(SUB, NL), (BL, 1)))
    thcm16_tl = np.ascontiguousarray(np.tile(thcm16.reshape(SUB, NL), (BL, 1)))

    nc = _get_nc()
    in_maps = [
        {
            "x": x[i * BL : (i + 1) * BL],
            "cm": cm_tl,
            "sigth": sigth_tl,
            "thcm16": thcm16_tl,
        }
        for i in range(NCORES)
    ]
    res = run_bass_kernel_spmd(
        nc, in_maps, core_ids=list(range(NCORES)), trace=_trace, tmpdir=_tmpdir
    )
    spikes = np.concatenate(
        [np.asarray(r["spikes8"]).astype(np.float32) for r in res.results],
        axis=0,
    )
    mems = np.concatenate(
        [np.asarray(r["mems16"]).astype(np.float32) for r in res.results],
        axis=0,
    )
    if _trace:
        _CACHE["last_results"] = res
    return spikes, mems


# revision 28
# speedup vs baseline: 1.0723x; 1.0723x over previous
"""LIF neuron scan kernel v4 for Trainium2 (8 NeuronCores).

Reference recurrence per timestep t (elementwise over B x N):
    u = (x_t - m)*sig + m ; s = (u >= th) ; m = (1-s)*u
with sig = sigmoid(tau_x) in (0,1), th > 0 per neuron.

uh-space (v3) reformulation kept: uh = u/th, xs = x*(sig/th), cm = 1-sig:
    w  = (uh < 1) * cm        # stt, fp32 (w==0 iff spike; w carries the decay)
    p  = uh * w               # tt fp32
    uh'= p + xs'              # tt fp32
State stays fp32 end-to-end: fp16-state variants flip ~2-3k near-threshold
spikes (emulated: 2.2-2.6e-2 rel err > the 2e-2 gate), so 16-bit state is
not usable; only non-state paths are 16-bit.

v4 changes vs v3 (probe-informed; see probe.py):
  - Input pipeline: one 2 MB dma_start per 8-t block, 3-slot ring with a
    per-slot semaphore (no cross-block issue throttle). v3 serialized input
    DMAs behind output waits + a completion throttle and DVE idled 4.8 us
    per block waiting for x.
  - mems path: ACT casts the p block to fp16 (ACT is ~70% idle), DVE does
    mems16 = p16 * thcm16 as an all-fp16 tensor_tensor -> 2x_1P mode
    (2.2 us vs 4.3 us per block). mems output fp16 (was bf16): same bytes,
    ~8x better mantissa. Accuracy impact ~5e-4 rel, well inside the gate.
  - spikes: one ACT Exp per block ([128,4096], fp8e4 out, exact 0/1),
    halving spike DMA bytes and cutting per-t semaphore traffic.
  - Chain ops carry no per-t then_inc except the block-final ones: pure TT
    pitch is 602 ns; v3's per-op sem traffic ran it at 722 ns.
  - GpSimd compute offload was probed and rejected: a gpsimd tensor_tensor
    blocks concurrent DVE almost completely (one DVE op per gpsimd op).
    SWDGE DMA traffic does NOT block DVE (probed) but is not needed.
  - DMA accum_op=mult (CCE) is rejected by the compiler ("DMACopy does not
    support mult with Copy mode") - multiply-during-DMA is not available.

Sharding: data-parallel over batch B across 8 cores (4 batches/core),
constants replicated; cores fully independent (recurrence is only over T).
Per-core layout: [128, 512] per timestep, partition p = b_local*32 + sub,
free = n_low, neuron n = sub*512 + n_low.
"""

import sys

if "/opt/trn_rl_repo" not in sys.path:
    sys.path.insert(0, "/opt/trn_rl_repo")

import contextlib

import numpy as np

import concourse.bass as bass
import concourse.mybir as mybir
from concourse.bass_utils import run_bass_kernel_spmd

B, T, N = 32, 64, 16384
NCORES = 8
BL = B // NCORES
SUB = 32
NL = N // SUB  # 512
P = BL * SUB  # 128
TBLK = 8
NBLK = T // TBLK
BW = TBLK * NL  # 4096
F32 = mybir.dt.float32
F16 = mybir.dt.float16
F8 = mybir.dt.float8e4
ALU = mybir.AluOpType
AF = mybir.ActivationFunctionType

_CACHE: dict = {}


def _build_nc() -> bass.Bass:
    nc = bass.Bass()
    x = nc.dram_tensor("x", [BL, T, N], F32, kind="ExternalInput")
    # host-pretiled consts: [128, 512], partition p=(b,s) -> neuron s*512+n
    cm_d = nc.dram_tensor("cm", [P, NL], F32, kind="ExternalInput")
    sigth_d = nc.dram_tensor("sigth", [P, NL], F32, kind="ExternalInput")
    thcm_d = nc.dram_tensor("thcm16", [P, NL], F16, kind="ExternalInput")
    spikes8 = nc.dram_tensor("spikes8", [BL, T, N], F8, kind="ExternalOutput")
    mems16 = nc.dram_tensor("mems16", [BL, T, N], F16, kind="ExternalOutput")

    def x_src(b, k):
        return x[b, k * TBLK : (k + 1) * TBLK, :].rearrange(
            "t (s n) -> s t n", n=NL
        )

    def out_dst(dram, b, k):
        return dram[b, k * TBLK : (k + 1) * TBLK, :].rearrange(
            "t (s n) -> s t n", n=NL
        )

    def bv(tile, b):
        return tile[b * SUB : (b + 1) * SUB, :].rearrange(
            "p (t n) -> p t n", n=NL
        )

    with contextlib.ExitStack() as st:
        xb_all = st.enter_context(nc.sbuf_tensor([P, 3 * BW], F32))
        xs_t = st.enter_context(nc.sbuf_tensor([P, BW], F32))
        sigthb = st.enter_context(nc.sbuf_tensor([P, BW], F32))
        thcmb16 = st.enter_context(nc.sbuf_tensor([P, BW], F16))
        cm_t = st.enter_context(nc.sbuf_tensor([P, NL], F32))
        sigth_t = st.enter_context(nc.sbuf_tensor([P, NL], F32))
        thcm16_t = st.enter_context(nc.sbuf_tensor([P, NL], F16))
        uh_t = st.enter_context(nc.sbuf_tensor([P, 2 * NL], F32))
        w_all = st.enter_context(nc.sbuf_tensor([P, 2 * BW], F32))
        p_all = st.enter_context(nc.sbuf_tensor([P, 2 * BW], F32))
        p16_t = st.enter_context(nc.sbuf_tensor([P, BW], F16))
        m16_all = st.enter_context(nc.sbuf_tensor([P, 2 * BW], F16))
        s8_t = st.enter_context(nc.sbuf_tensor([P, BW], F8))
        c_sem = st.enter_context(nc.semaphore("c_sem"))
        rep_sem = st.enter_context(nc.semaphore("rep_sem"))
        xs0_sem = st.enter_context(nc.semaphore("xs0_sem"))
        xs1_sem = st.enter_context(nc.semaphore("xs1_sem"))
        xs2_sem = st.enter_context(nc.semaphore("xs2_sem"))
        xsd_sem = st.enter_context(nc.semaphore("xsd_sem"))
        w_sem = st.enter_context(nc.semaphore("w_sem"))
        pb_sem = st.enter_context(nc.semaphore("pb_sem"))
        p16_sem = st.enter_context(nc.semaphore("p16_sem"))
        spk_sem = st.enter_context(nc.semaphore("spk_sem"))
        m16d_sem = st.enter_context(nc.semaphore("m16d_sem"))
        mo_sem = st.enter_context(nc.semaphore("mo_sem"))
        so_sem = st.enter_context(nc.semaphore("so_sem"))
        block = st.enter_context(nc.Block())

        xslot_sems = [xs0_sem, xs1_sem, xs2_sem]

        def xb_r(k):
            return xb_all[:, (k % 3) * BW : (k % 3 + 1) * BW]

        def wsl(k, tl):
            r = k % 2
            return w_all[:, (r * TBLK + tl) * NL : (r * TBLK + tl + 1) * NL]

        def wblk(k):
            r = k % 2
            return w_all[:, r * BW : (r + 1) * BW]

        def psl(k, tl):
            r = k % 2
            return p_all[:, (r * TBLK + tl) * NL : (r * TBLK + tl + 1) * NL]

        def pblk(k):
            r = k % 2
            return p_all[:, r * BW : (r + 1) * BW]

        def uhsl(t):
            r = t % 2
            return uh_t[:, r * NL : (r + 1) * NL]

        def m16sl(j):
            r = j % 2
            return m16_all[:, r * BW : (r + 1) * BW]

        @block.sync
        def _(sync):
            # consts first (tiny, unblock ACT replication), then x0 (b=0,1;
            # the other half is issued from the ACT queue), then x1/x2
            for src, dst in (
                (sigth_d, sigth_t),
                (cm_d, cm_t),
                (thcm_d, thcm16_t),
            ):
                sync.dma_start(out=dst[:, :], in_=src[:, :]).then_inc(c_sem, 16)
            for b in (0, 1):
                sync.dma_start(out=bv(xb_r(0), b), in_=x_src(b, 0)).then_inc(
                    xslot_sems[0], 16
                )
            # x1/x2 wait for x0 to land so they don't steal its bandwidth
            sync.wait_ge(xslot_sems[0], 64)
            for k in (1, 2):
                for b in range(BL):
                    sync.dma_start(out=bv(xb_r(k), b), in_=x_src(b, k)).then_inc(
                        xslot_sems[k % 3], 16
                    )
            # mid-loop x input DMAs are issued from the ACT queue (the other
            # HWDGE ring) so they never serialize behind the output waits here.
            # mems before spikes: m16d clears mid-block, spk only at block end.
            for k in range(NBLK):
                if k >= 1:
                    sync.wait_ge(m16d_sem, k)
                    for b in range(BL):
                        sync.dma_start(
                            out=out_dst(mems16, b, k - 1),
                            in_=bv(m16sl(k - 1), b),
                        ).then_inc(mo_sem, 16)
                if k < NBLK - 1:
                    sync.wait_ge(spk_sem, k + 1)
                    for b in range(BL):
                        sync.dma_start(
                            out=out_dst(spikes8, b, k), in_=bv(s8_t, b)
                        ).then_inc(so_sem, 16)
            # tail: last mems (ready at chain end) before last spikes (ready
            # one exp later)
            sync.wait_ge(m16d_sem, NBLK)
            for b in range(BL):
                sync.dma_start(
                    out=out_dst(mems16, b, NBLK - 1),
                    in_=bv(m16sl(NBLK - 1), b),
                ).then_inc(mo_sem, 16)
            sync.wait_ge(spk_sem, NBLK)
            for b in range(BL):
                sync.dma_start(
                    out=out_dst(spikes8, b, NBLK - 1), in_=bv(s8_t, b)
                ).then_inc(so_sem, 16)
            sync.wait_ge(so_sem, 64 * NBLK)
            sync.wait_ge(mo_sem, 64 * NBLK)

        @block.vector
        def _(vector):
            vector.wait_ge(c_sem, 32)  # cm_t loaded
            vector.wait_ge(rep_sem, TBLK)  # sigthb tiled
            for k in range(NBLK):
                vector.wait_ge(xslot_sems[k % 3], 64 * (k // 3 + 1))
                if k >= 2:
                    # p ring slot k%2: ACT cast of block k-2 must be done
                    vector.wait_ge(p16_sem, k - 1)
                # xs for block k
                nc.vector.tensor_tensor(
                    out=xs_t[:, :], in0=xb_r(k), in1=sigthb[:, :], op=ALU.mult
                ).then_inc(xsd_sem, 1)
                if k >= 1:
                    # deferred add: uh_{8k} = p_{8k-1} + xs_{8k}
                    nc.vector.tensor_tensor(
                        out=uhsl(8 * k),
                        in0=psl(k - 1, TBLK - 1),
                        in1=xs_t[:, 0:NL],
                        op=ALU.add,
                    )
                if k >= 2:
                    vector.wait_ge(spk_sem, k - 1)  # w ring WAR vs ACT exp
                for tl in range(TBLK):
                    t = k * TBLK + tl
                    if k >= 1 and tl == 4:
                        # mems16 for block k-1, placed mid-chain so the ACT
                        # cast (done ~7us after chain k-1) is never waited on
                        vector.wait_ge(p16_sem, k)
                        if k >= 3:
                            vector.wait_ge(mo_sem, 64 * (k - 2))  # m16 WAR
                        nc.vector.tensor_tensor(
                            out=m16sl(k - 1),
                            in0=p16_t[:, :],
                            in1=thcmb16[:, :],
                            op=ALU.mult,
                        ).then_inc(m16d_sem, 1)
                    uh = xs_t[:, 0:NL] if t == 0 else uhsl(t)
                    ins_w = nc.vector.scalar_tensor_tensor(
                        out=wsl(k, tl),
                        in0=uh,
                        scalar=1.0,
                        in1=cm_t[:, :],
                        op0=ALU.is_lt,
                        op1=ALU.mult,
                    )
                    if tl == TBLK - 1:
                        ins_w.then_inc(w_sem, 1)
                    ins_p = nc.vector.tensor_tensor(
                        out=psl(k, tl), in0=uh, in1=wsl(k, tl), op=ALU.mult
                    )
                    if tl == TBLK - 1:
                        ins_p.then_inc(pb_sem, 1)
                    if k == NBLK - 1:
                        # last block: mems per-t straight from fp32 p (mixed
                        # dtype TT) so the final mems DMA starts immediately
                        if tl == 0:
                            vector.wait_ge(mo_sem, 64 * (NBLK - 2))
                        ins_m = nc.vector.tensor_tensor(
                            out=m16sl(k)[:, tl * NL : (tl + 1) * NL],
                            in0=psl(k, tl),
                            in1=thcmb16[:, tl * NL : (tl + 1) * NL],
                            op=ALU.mult,
                        )
                        if tl == TBLK - 1:
                            ins_m.then_inc(m16d_sem, 1)
                    if tl < TBLK - 1:
                        nc.vector.tensor_tensor(
                            out=uhsl(t + 1),
                            in0=psl(k, tl),
                            in1=xs_t[:, (tl + 1) * NL : (tl + 2) * NL],
                            op=ALU.add,
                        )

        @block.scalar
        def _(scalar):
            # other half of x block 0 (parallel issue with the sync queue)
            for b in (2, 3):
                nc.scalar.dma_start(
                    out=bv(xb_r(0), b), in_=x_src(b, 0)
                ).then_inc(xslot_sems[0], 16)
            scalar.wait_ge(c_sem, 16)  # sigth_t loaded
            for tl in range(TBLK):
                nc.scalar.copy(
                    out=sigthb[:, tl * NL : (tl + 1) * NL], in_=sigth_t[:, :]
                ).then_inc(rep_sem, 1)
            scalar.wait_ge(c_sem, 48)
            for tl in range(TBLK):
                nc.scalar.copy(
                    out=thcmb16[:, tl * NL : (tl + 1) * NL], in_=thcm16_t[:, :]
                ).then_inc(rep_sem, 1)
            for k in range(NBLK):
                # spikes block k first (gates DVE's w ring + sync's s8-out):
                # w==0 iff spike; exp(-1e30*w) = 1/0 exactly
                scalar.wait_ge(w_sem, k + 1)
                if k >= 1:
                    scalar.wait_ge(so_sem, 64 * k)  # s8 WAR
                nc.scalar.activation(
                    s8_t[:, :], wblk(k), AF.Exp, scale=-1e30
                ).then_inc(spk_sem, 1)
                # issue x input DMA for block k+3 (other HWDGE ring; gated
                # only on the xs-op that frees the ring slot)
                kf = k + 3
                if kf < NBLK:
                    scalar.wait_ge(xsd_sem, k + 1)
                    for b in range(BL):
                        nc.scalar.dma_start(
                            out=bv(xb_r(kf), b), in_=x_src(b, kf)
                        ).then_inc(xslot_sems[kf % 3], 16)
                # p block k -> fp16 (for the all-16-bit mems mult); skipped
                # for the last block (its mems come straight from fp32 p)
                if k < NBLK - 1:
                    scalar.wait_ge(pb_sem, k + 1)
                    if k >= 1:
                        scalar.wait_ge(m16d_sem, k)  # p16 used by m16-op k-1
                    nc.scalar.copy(out=p16_t[:, :], in_=pblk(k)).then_inc(
                        p16_sem, 1
                    )

    return nc


def _get_nc() -> bass.Bass:
    if "nc" not in _CACHE:
        _CACHE["nc"] = _build_nc()
    return _CACHE["nc"]


def kernel(x, thresh, tau_x, _trace: bool = False, _tmpdir: str | None = None):
    x = np.ascontiguousarray(np.asarray(x, dtype=np.float32))
    thresh = np.ascontiguousarray(np.asarray(thresh, dtype=np.float32))
    tau_x = np.ascontiguousarray(np.asarray(tau_x, dtype=np.float32))
    assert x.shape == (B, T, N)

    # O(N) host-side constants; all O(B*T*N) math happens on-device.
    sig = (1.0 / (1.0 + np.exp(-tau_x.astype(np.float64)))).astype(np.float32)
    cm = (np.float32(1.0) - sig).astype(np.float32)
    sigth = (sig / thresh).astype(np.float32)
    thcm16 = (thresh / cm).astype(np.float16)
    # pretile to [128, 512]: partition p = b_local*32 + s holds neuron
    # chunk s; replicate the [32, 512] view across the 4 b-groups
    cm_tl = np.ascontiguousarray(np.tile(cm.reshape(SUB, NL), (BL, 1)))
    sigth_tl = np.ascontiguousarray(np.tile(sigth.reshape(SUB, NL), (BL, 1)))
    thcm16_tl = np.ascontiguousarray(np.tile(thcm16.reshape(SUB, NL), (BL, 1)))

    nc = _get_nc()
    in_maps = [
        {
            "x": x[i * BL : (i + 1) * BL],
            "cm": cm_tl,
            "sigth": sigth_tl,
            "thcm16": thcm16_tl,
        }
        for i in range(NCORES)
    ]
    res = run_bass_kernel_spmd(
        nc, in_maps, core_ids=list(range(NCORES)), trace=_trace, tmpdir=_tmpdir
    )
    spikes = np.concatenate(
        [np.asarray(r["spikes8"]).astype(np.float32) for r in res.results],
        axis=0,
    )
    mems = np.concatenate(
        [np.asarray(r["mems16"]).astype(np.float32) for r in res.results],
        axis=0,
    )
    if _trace:
        _CACHE["last_results"] = res
    return spikes, mems


# revision 41
# speedup vs baseline: 1.0854x; 1.0122x over previous
"""LIF neuron scan kernel v4 for Trainium2 (8 NeuronCores).

Reference recurrence per timestep t (elementwise over B x N):
    u = (x_t - m)*sig + m ; s = (u >= th) ; m = (1-s)*u
with sig = sigmoid(tau_x) in (0,1), th > 0 per neuron.

uh-space (v3) reformulation kept: uh = u/th, xs = x*(sig/th), cm = 1-sig:
    w  = (uh < 1) * cm        # stt, fp32 (w==0 iff spike; w carries the decay)
    p  = uh * w               # tt fp32
    uh'= p + xs'              # tt fp32
State stays fp32 end-to-end: fp16-state variants flip ~2-3k near-threshold
spikes (emulated: 2.2-2.6e-2 rel err > the 2e-2 gate), so 16-bit state is
not usable; only non-state paths are 16-bit.

v4 changes vs v3 (probe-informed; see probe.py):
  - Input pipeline: one 2 MB dma_start per 8-t block, 3-slot ring with a
    per-slot semaphore (no cross-block issue throttle). v3 serialized input
    DMAs behind output waits + a completion throttle and DVE idled 4.8 us
    per block waiting for x.
  - mems path: ACT casts the p block to fp16 (ACT is ~70% idle), DVE does
    mems16 = p16 * thcm16 as an all-fp16 tensor_tensor -> 2x_1P mode
    (2.2 us vs 4.3 us per block). mems output fp16 (was bf16): same bytes,
    ~8x better mantissa. Accuracy impact ~5e-4 rel, well inside the gate.
  - spikes: one ACT Exp per block ([128,4096], fp8e4 out, exact 0/1),
    halving spike DMA bytes and cutting per-t semaphore traffic.
  - Chain ops carry no per-t then_inc except the block-final ones: pure TT
    pitch is 602 ns; v3's per-op sem traffic ran it at 722 ns.
  - GpSimd compute offload was probed and rejected: a gpsimd tensor_tensor
    blocks concurrent DVE almost completely (one DVE op per gpsimd op).
    SWDGE DMA traffic does NOT block DVE (probed) but is not needed.
  - DMA accum_op=mult (CCE) is rejected by the compiler ("DMACopy does not
    support mult with Copy mode") - multiply-during-DMA is not available.

Sharding: data-parallel over batch B across 8 cores (4 batches/core),
constants replicated; cores fully independent (recurrence is only over T).
Per-core layout: [128, 512] per timestep, partition p = b_local*32 + sub,
free = n_low, neuron n = sub*512 + n_low.
"""

import sys

if "/opt/trn_rl_repo" not in sys.path:
    sys.path.insert(0, "/opt/trn_rl_repo")

import contextlib

import numpy as np

import concourse.bass as bass
import concourse.mybir as mybir
from concourse.bass_utils import run_bass_kernel_spmd

B, T, N = 32, 64, 16384
NCORES = 8
BL = B // NCORES
SUB = 32
NL = N // SUB  # 512
P = BL * SUB  # 128
TBLK = 8
NBLK = T // TBLK
BW = TBLK * NL  # 4096
F32 = mybir.dt.float32
F16 = mybir.dt.float16
F8 = mybir.dt.float8e4
ALU = mybir.AluOpType
AF = mybir.ActivationFunctionType

_CACHE: dict = {}


def _build_nc() -> bass.Bass:
    nc = bass.Bass()
    x = nc.dram_tensor("x", [BL, T, N], F32, kind="ExternalInput")
    # host-pretiled consts: [128, 512], partition p=(b,s) -> neuron s*512+n
    cm_d = nc.dram_tensor("cm", [P, NL], F32, kind="ExternalInput")
    sigth_d = nc.dram_tensor("sigth", [P, NL], F32, kind="ExternalInput")
    thcm_d = nc.dram_tensor("thcm16", [P, NL], F16, kind="ExternalInput")
    spikes8 = nc.dram_tensor("spikes8", [BL, T, N], F8, kind="ExternalOutput")
    mems16 = nc.dram_tensor("mems16", [BL, T, N], F16, kind="ExternalOutput")

    def x_src(b, k):
        return x[b, k * TBLK : (k + 1) * TBLK, :].rearrange(
            "t (s n) -> s t n", n=NL
        )

    def out_dst(dram, b, k):
        return dram[b, k * TBLK : (k + 1) * TBLK, :].rearrange(
            "t (s n) -> s t n", n=NL
        )

    def out_dst_h(dram, b, k, h):
        t0 = k * TBLK + h * (TBLK // 2)
        return dram[b, t0 : t0 + TBLK // 2, :].rearrange(
            "t (s n) -> s t n", n=NL
        )

    def bv_h(tile, b, h):
        half = BW // 2
        return tile[b * SUB : (b + 1) * SUB, h * half : (h + 1) * half].rearrange(
            "p (t n) -> p t n", n=NL
        )

    def bv(tile, b):
        return tile[b * SUB : (b + 1) * SUB, :].rearrange(
            "p (t n) -> p t n", n=NL
        )

    with contextlib.ExitStack() as st:
        xb_all = st.enter_context(nc.sbuf_tensor([P, 3 * BW], F32))
        xs_t = st.enter_context(nc.sbuf_tensor([P, BW], F32))
        sigthb = st.enter_context(nc.sbuf_tensor([P, BW], F32))
        thcmb16 = st.enter_context(nc.sbuf_tensor([P, BW], F16))
        cm_t = st.enter_context(nc.sbuf_tensor([P, NL], F32))
        sigth_t = st.enter_context(nc.sbuf_tensor([P, NL], F32))
        thcm16_t = st.enter_context(nc.sbuf_tensor([P, NL], F16))
        uh_t = st.enter_context(nc.sbuf_tensor([P, 2 * NL], F32))
        w_all = st.enter_context(nc.sbuf_tensor([P, 2 * BW], F32))
        p_all = st.enter_context(nc.sbuf_tensor([P, 2 * BW], F32))
        p16_t = st.enter_context(nc.sbuf_tensor([P, BW], F16))
        m16_all = st.enter_context(nc.sbuf_tensor([P, 2 * BW], F16))
        s8_t = st.enter_context(nc.sbuf_tensor([P, BW], F8))
        c_sem = st.enter_context(nc.semaphore("c_sem"))
        sg_sem = st.enter_context(nc.semaphore("sg_sem"))
        rep_sem = st.enter_context(nc.semaphore("rep_sem"))
        xs0_sem = st.enter_context(nc.semaphore("xs0_sem"))
        xs1_sem = st.enter_context(nc.semaphore("xs1_sem"))
        xs2_sem = st.enter_context(nc.semaphore("xs2_sem"))
        xsd_sem = st.enter_context(nc.semaphore("xsd_sem"))
        w_sem = st.enter_context(nc.semaphore("w_sem"))
        pb_sem = st.enter_context(nc.semaphore("pb_sem"))
        p16_sem = st.enter_context(nc.semaphore("p16_sem"))
        spk_sem = st.enter_context(nc.semaphore("spk_sem"))
        m16d_sem = st.enter_context(nc.semaphore("m16d_sem"))
        m7_sem = st.enter_context(nc.semaphore("m7_sem"))
        mo_sem = st.enter_context(nc.semaphore("mo_sem"))
        so_sem = st.enter_context(nc.semaphore("so_sem"))
        block = st.enter_context(nc.Block())

        xslot_sems = [xs0_sem, xs1_sem, xs2_sem]

        def xb_r(k):
            return xb_all[:, (k % 3) * BW : (k % 3 + 1) * BW]

        def wsl(k, tl):
            r = k % 2
            return w_all[:, (r * TBLK + tl) * NL : (r * TBLK + tl + 1) * NL]

        def wblk(k):
            r = k % 2
            return w_all[:, r * BW : (r + 1) * BW]

        def psl(k, tl):
            r = k % 2
            return p_all[:, (r * TBLK + tl) * NL : (r * TBLK + tl + 1) * NL]

        def pblk(k):
            r = k % 2
            return p_all[:, r * BW : (r + 1) * BW]

        def uhsl(t):
            r = t % 2
            return uh_t[:, r * NL : (r + 1) * NL]

        def m16sl(j):
            r = j % 2
            return m16_all[:, r * BW : (r + 1) * BW]

        @block.sync
        def _(sync):
            # consts first (tiny, unblock ACT replication), then x0 (b=0,1;
            # sigth + the other x0 half are issued from the ACT queue)
            for src, dst in ((cm_d, cm_t), (thcm_d, thcm16_t)):
                sync.dma_start(out=dst[:, :], in_=src[:, :]).then_inc(c_sem, 16)
            for b in (0, 1):
                sync.dma_start(out=bv(xb_r(0), b), in_=x_src(b, 0)).then_inc(
                    xslot_sems[0], 16
                )
            # x1/x2 wait for x0 to land so they don't steal its bandwidth
            sync.wait_ge(xslot_sems[0], 64)
            for k in (1, 2):
                for b in range(BL):
                    sync.dma_start(out=bv(xb_r(k), b), in_=x_src(b, k)).then_inc(
                        xslot_sems[k % 3], 16
                    )
            # mid-loop x input DMAs are issued from the ACT queue (the other
            # HWDGE ring) so they never serialize behind the output waits here.
            # mems before spikes: m16d clears mid-block, spk only at block end.
            for k in range(NBLK):
                if k >= 1:
                    sync.wait_ge(m16d_sem, k)
                    for b in range(BL):
                        sync.dma_start(
                            out=out_dst(mems16, b, k - 1),
                            in_=bv(m16sl(k - 1), b),
                        ).then_inc(mo_sem, 16)
                if k < NBLK - 1:
                    sync.wait_ge(spk_sem, k + 1)
                    for b in range(BL):
                        sync.dma_start(
                            out=out_dst(spikes8, b, k), in_=bv(s8_t, b)
                        ).then_inc(so_sem, 16)
            # tail: halved outputs of the last block, interleaved by readiness
            # (mems-half h at m16d >= 8+h, spikes-half h at spk >= 8+h)
            kl = NBLK - 1
            for h in (0, 1):
                sync.wait_ge(m7_sem, h + 1)
                for b in range(BL):
                    sync.dma_start(
                        out=out_dst_h(mems16, b, kl, h),
                        in_=bv_h(m16sl(kl), b, h),
                    ).then_inc(mo_sem, 16)
                sync.wait_ge(spk_sem, NBLK + h)
                for b in range(BL):
                    sync.dma_start(
                        out=out_dst_h(spikes8, b, kl, h), in_=bv_h(s8_t, b, h)
                    ).then_inc(so_sem, 16)
            sync.wait_ge(so_sem, 64 * (NBLK - 1) + 128)
            sync.wait_ge(mo_sem, 64 * (NBLK - 1) + 128)

        @block.vector
        def _(vector):
            vector.wait_ge(c_sem, 32)  # cm_t + thcm16_t loaded
            vector.wait_ge(rep_sem, TBLK)  # sigthb tiled
            for k in range(NBLK):
                vector.wait_ge(xslot_sems[k % 3], 64 * (k // 3 + 1))
                if k >= 2:
                    # p ring slot k%2: ACT cast of block k-2 must be done
                    vector.wait_ge(p16_sem, k - 1)
                # xs for block k
                nc.vector.tensor_tensor(
                    out=xs_t[:, :], in0=xb_r(k), in1=sigthb[:, :], op=ALU.mult
                ).then_inc(xsd_sem, 1)
                if k >= 1:
                    # deferred add: uh_{8k} = p_{8k-1} + xs_{8k}
                    nc.vector.tensor_tensor(
                        out=uhsl(8 * k),
                        in0=psl(k - 1, TBLK - 1),
                        in1=xs_t[:, 0:NL],
                        op=ALU.add,
                    )
                if k >= 2:
                    vector.wait_ge(spk_sem, k - 1)  # w ring WAR vs ACT exp
                for tl in range(TBLK):
                    t = k * TBLK + tl
                    if k >= 1 and tl == 4:
                        # mems16 for block k-1, placed mid-chain so the ACT
                        # cast (done ~7us after chain k-1) is never waited on
                        vector.wait_ge(p16_sem, k)
                        if k >= 3:
                            vector.wait_ge(mo_sem, 64 * (k - 2))  # m16 WAR
                        nc.vector.tensor_tensor(
                            out=m16sl(k - 1),
                            in0=p16_t[:, :],
                            in1=thcmb16[:, :],
                            op=ALU.mult,
                        ).then_inc(m16d_sem, 1)
                    uh = xs_t[:, 0:NL] if t == 0 else uhsl(t)
                    ins_w = nc.vector.scalar_tensor_tensor(
                        out=wsl(k, tl),
                        in0=uh,
                        scalar=1.0,
                        in1=cm_t[:, :],
                        op0=ALU.is_lt,
                        op1=ALU.mult,
                    )
                    if tl == TBLK - 1 or (
                        k == NBLK - 1 and tl == TBLK // 2 - 1
                    ):
                        ins_w.then_inc(w_sem, 1)
                    ins_p = nc.vector.tensor_tensor(
                        out=psl(k, tl), in0=uh, in1=wsl(k, tl), op=ALU.mult
                    )
                    if tl == TBLK - 1:
                        ins_p.then_inc(pb_sem, 1)
                    if k == NBLK - 1:
                        # last block: mems per-t straight from fp32 p (mixed
                        # dtype TT) so the final mems DMA starts immediately
                        if tl == 0:
                            vector.wait_ge(mo_sem, 64 * (NBLK - 2))
                        ins_m = nc.vector.tensor_tensor(
                            out=m16sl(k)[:, tl * NL : (tl + 1) * NL],
                            in0=psl(k, tl),
                            in1=thcmb16[:, tl * NL : (tl + 1) * NL],
                            op=ALU.mult,
                        )
                        if tl in (TBLK // 2 - 1, TBLK - 1):
                            ins_m.then_inc(m7_sem, 1)
                    if tl < TBLK - 1:
                        nc.vector.tensor_tensor(
                            out=uhsl(t + 1),
                            in0=psl(k, tl),
                            in1=xs_t[:, (tl + 1) * NL : (tl + 2) * NL],
                            op=ALU.add,
                        )

        @block.scalar
        def _(scalar):
            # sigth + other half of x block 0 (parallel with the sync queue)
            nc.scalar.dma_start(out=sigth_t[:, :], in_=sigth_d[:, :]).then_inc(
                sg_sem, 16
            )
            for b in (2, 3):
                nc.scalar.dma_start(
                    out=bv(xb_r(0), b), in_=x_src(b, 0)
                ).then_inc(xslot_sems[0], 16)
            scalar.wait_ge(sg_sem, 16)  # sigth_t loaded
            for tl in range(TBLK):
                nc.scalar.copy(
                    out=sigthb[:, tl * NL : (tl + 1) * NL], in_=sigth_t[:, :]
                ).then_inc(rep_sem, 1)
            scalar.wait_ge(c_sem, 32)
            for tl in range(TBLK):
                nc.scalar.copy(
                    out=thcmb16[:, tl * NL : (tl + 1) * NL], in_=thcm16_t[:, :]
                ).then_inc(rep_sem, 1)
            for k in range(NBLK):
                # spikes block k first (gates DVE's w ring + sync's s8-out):
                # w==0 iff spike; exp(-1e30*w) = 1/0 exactly
                if k < NBLK - 1:
                    scalar.wait_ge(w_sem, k + 1)
                    if k >= 1:
                        scalar.wait_ge(so_sem, 64 * k)  # s8 WAR
                    nc.scalar.activation(
                        s8_t[:, :], wblk(k), AF.Exp, scale=-1e30
                    ).then_inc(spk_sem, 1)
                else:
                    # last block: spikes in halves so output DMA starts early
                    scalar.wait_ge(so_sem, 64 * k)
                    half = BW // 2
                    for h in (0, 1):
                        scalar.wait_ge(w_sem, k + 1 + h)
                        nc.scalar.activation(
                            s8_t[:, h * half : (h + 1) * half],
                            wblk(k)[:, h * half : (h + 1) * half],
                            AF.Exp,
                            scale=-1e30,
                        ).then_inc(spk_sem, 1)
                # issue x input DMA for block k+3 (other HWDGE ring; gated
                # only on the xs-op that frees the ring slot)
                kf = k + 3
                if kf < NBLK:
                    scalar.wait_ge(xsd_sem, k + 1)
                    for b in range(BL):
                        nc.scalar.dma_start(
                            out=bv(xb_r(kf), b), in_=x_src(b, kf)
                        ).then_inc(xslot_sems[kf % 3], 16)
                # p block k -> fp16 (for the all-16-bit mems mult); skipped
                # for the last block (its mems come straight from fp32 p)
                if k < NBLK - 1:
                    scalar.wait_ge(pb_sem, k + 1)
                    if k >= 1:
                        scalar.wait_ge(m16d_sem, k)  # p16 used by m16-op k-1
                    nc.scalar.copy(out=p16_t[:, :], in_=pblk(k)).then_inc(
                        p16_sem, 1
                    )

    return nc


def _get_nc() -> bass.Bass:
    if "nc" not in _CACHE:
        _CACHE["nc"] = _build_nc()
    return _CACHE["nc"]


def kernel(x, thresh, tau_x, _trace: bool = False, _tmpdir: str | None = None):
    x = np.ascontiguousarray(np.asarray(x, dtype=np.float32))
    thresh = np.ascontiguousarray(np.asarray(thresh, dtype=np.float32))
    tau_x = np.ascontiguousarray(np.asarray(tau_x, dtype=np.float32))
    assert x.shape == (B, T, N)

    # O(N) host-side constants; all O(B*T*N) math happens on-device.
    sig = (1.0 / (1.0 + np.exp(-tau_x.astype(np.float64)))).astype(np.float32)
    cm = (np.float32(1.0) - sig).astype(np.float32)
    sigth = (sig / thresh).astype(np.float32)
    thcm16 = (thresh / cm).astype(np.float16)
    # pretile to [128, 512]: partition p = b_local*32 + s holds neuron
    # chunk s; replicate the [32, 512] view across the 4 b-groups
    cm_tl = np.ascontiguousarray(np.tile(cm.reshape(SUB, NL), (BL, 1)))
    sigth_tl = np.ascontiguousarray(np.tile(sigth.reshape(SUB, NL), (BL, 1)))
    thcm16_tl = np.ascontiguousarray(np.tile(thcm16.reshape(SUB, NL), (BL, 1)))

    nc = _get_nc()
    in_maps = [
        {
            "x": x[i * BL : (i + 1) * BL],
            "cm": cm_tl,
            "sigth": sigth_tl,
            "thcm16": thcm16_tl,
        }
        for i in range(NCORES)
    ]
    res = run_bass_kernel_spmd(
        nc, in_maps, core_ids=list(range(NCORES)), trace=_trace, tmpdir=_tmpdir
    )
    spikes = np.concatenate(
        [np.asarray(r["spikes8"]).astype(np.float32) for r in res.results],
        axis=0,
    )
    mems = np.concatenate(
        [np.asarray(r["mems16"]).astype(np.float32) for r in res.results],
        axis=0,
    )
    if _trace:
        _CACHE["last_results"] = res
    return spikes, mems


# revision 48
# speedup vs baseline: 1.0888x; 1.0031x over previous
"""LIF neuron scan kernel v4 for Trainium2 (8 NeuronCores).

Reference recurrence per timestep t (elementwise over B x N):
    u = (x_t - m)*sig + m ; s = (u >= th) ; m = (1-s)*u
with sig = sigmoid(tau_x) in (0,1), th > 0 per neuron.

uh-space (v3) reformulation kept: uh = u/th, xs = x*(sig/th), cm = 1-sig:
    w  = (uh < 1) * cm        # stt, fp32 (w==0 iff spike; w carries the decay)
    p  = uh * w               # tt fp32
    uh'= p + xs'              # tt fp32
State stays fp32 end-to-end: fp16-state variants flip ~2-3k near-threshold
spikes (emulated: 2.2-2.6e-2 rel err > the 2e-2 gate), so 16-bit state is
not usable; only non-state paths are 16-bit.

v4 changes vs v3 (probe-informed; see probe.py):
  - Input pipeline: one 2 MB dma_start per 8-t block, 3-slot ring with a
    per-slot semaphore (no cross-block issue throttle). v3 serialized input
    DMAs behind output waits + a completion throttle and DVE idled 4.8 us
    per block waiting for x.
  - mems path: ACT casts the p block to fp16 (ACT is ~70% idle), DVE does
    mems16 = p16 * thcm16 as an all-fp16 tensor_tensor -> 2x_1P mode
    (2.2 us vs 4.3 us per block). mems output fp16 (was bf16): same bytes,
    ~8x better mantissa. Accuracy impact ~5e-4 rel, well inside the gate.
  - spikes: one ACT Exp per block ([128,4096], fp8e4 out, exact 0/1),
    halving spike DMA bytes and cutting per-t semaphore traffic.
  - Chain ops carry no per-t then_inc except the block-final ones: pure TT
    pitch is 602 ns; v3's per-op sem traffic ran it at 722 ns.
  - GpSimd compute offload was probed and rejected: a gpsimd tensor_tensor
    blocks concurrent DVE almost completely (one DVE op per gpsimd op).
    SWDGE DMA traffic does NOT block DVE (probed) but is not needed.
  - DMA accum_op=mult (CCE) is rejected by the compiler ("DMACopy does not
    support mult with Copy mode") - multiply-during-DMA is not available.

Sharding: data-parallel over batch B across 8 cores (4 batches/core),
constants replicated; cores fully independent (recurrence is only over T).
Per-core layout: [128, 512] per timestep, partition p = b_local*32 + sub,
free = n_low, neuron n = sub*512 + n_low.
"""

import sys

if "/opt/trn_rl_repo" not in sys.path:
    sys.path.insert(0, "/opt/trn_rl_repo")

import contextlib

import numpy as np

import concourse.bass as bass
import concourse.mybir as mybir
from concourse.bass_utils import run_bass_kernel_spmd

B, T, N = 32, 64, 16384
NCORES = 8
BL = B // NCORES
SUB = 32
NL = N // SUB  # 512
P = BL * SUB  # 128
TBLK = 8
NBLK = T // TBLK
BW = TBLK * NL  # 4096
F32 = mybir.dt.float32
F16 = mybir.dt.float16
F8 = mybir.dt.float8e4
ALU = mybir.AluOpType
AF = mybir.ActivationFunctionType

_CACHE: dict = {}


def _build_nc() -> bass.Bass:
    nc = bass.Bass()
    x = nc.dram_tensor("x", [BL, T, N], F32, kind="ExternalInput")
    # host-pretiled consts: [128, 512], partition p=(b,s) -> neuron s*512+n
    cm_d = nc.dram_tensor("cm", [P, NL], F32, kind="ExternalInput")
    sigth_d = nc.dram_tensor("sigth", [P, NL], F32, kind="ExternalInput")
    thcm_d = nc.dram_tensor("thcm16", [P, NL], F16, kind="ExternalInput")
    spikes8 = nc.dram_tensor("spikes8", [BL, T, N], F8, kind="ExternalOutput")
    mems16 = nc.dram_tensor("mems16", [BL, T, N], F16, kind="ExternalOutput")

    def x_src(b, k):
        return x[b, k * TBLK : (k + 1) * TBLK, :].rearrange(
            "t (s n) -> s t n", n=NL
        )

    def x_src_h(b, k, h):
        t0 = k * TBLK + h * (TBLK // 2)
        return x[b, t0 : t0 + TBLK // 2, :].rearrange(
            "t (s n) -> s t n", n=NL
        )

    def out_dst(dram, b, k):
        return dram[b, k * TBLK : (k + 1) * TBLK, :].rearrange(
            "t (s n) -> s t n", n=NL
        )

    def out_dst_h(dram, b, k, h):
        t0 = k * TBLK + h * (TBLK // 2)
        return dram[b, t0 : t0 + TBLK // 2, :].rearrange(
            "t (s n) -> s t n", n=NL
        )

    def bv_h(tile, b, h):
        half = BW // 2
        return tile[b * SUB : (b + 1) * SUB, h * half : (h + 1) * half].rearrange(
            "p (t n) -> p t n", n=NL
        )

    def bv(tile, b):
        return tile[b * SUB : (b + 1) * SUB, :].rearrange(
            "p (t n) -> p t n", n=NL
        )

    with contextlib.ExitStack() as st:
        xb_all = st.enter_context(nc.sbuf_tensor([P, 3 * BW], F32))
        xs_t = st.enter_context(nc.sbuf_tensor([P, BW], F32))
        sigthb = st.enter_context(nc.sbuf_tensor([P, BW], F32))
        thcmb16 = st.enter_context(nc.sbuf_tensor([P, BW], F16))
        cm_t = st.enter_context(nc.sbuf_tensor([P, NL], F32))
        sigth_t = st.enter_context(nc.sbuf_tensor([P, NL], F32))
        thcm16_t = st.enter_context(nc.sbuf_tensor([P, NL], F16))
        uh_t = st.enter_context(nc.sbuf_tensor([P, 2 * NL], F32))
        w_all = st.enter_context(nc.sbuf_tensor([P, 2 * BW], F32))
        p_all = st.enter_context(nc.sbuf_tensor([P, 2 * BW], F32))
        p16_t = st.enter_context(nc.sbuf_tensor([P, BW], F16))
        m16_all = st.enter_context(nc.sbuf_tensor([P, 2 * BW], F16))
        s8_t = st.enter_context(nc.sbuf_tensor([P, BW], F8))
        c_sem = st.enter_context(nc.semaphore("c_sem"))
        sg_sem = st.enter_context(nc.semaphore("sg_sem"))
        rep_sem = st.enter_context(nc.semaphore("rep_sem"))
        xh0_sem = st.enter_context(nc.semaphore("xh0_sem"))
        xs0_sem = st.enter_context(nc.semaphore("xs0_sem"))
        xs1_sem = st.enter_context(nc.semaphore("xs1_sem"))
        xs2_sem = st.enter_context(nc.semaphore("xs2_sem"))
        xsd_sem = st.enter_context(nc.semaphore("xsd_sem"))
        w_sem = st.enter_context(nc.semaphore("w_sem"))
        pb_sem = st.enter_context(nc.semaphore("pb_sem"))
        p16_sem = st.enter_context(nc.semaphore("p16_sem"))
        spk_sem = st.enter_context(nc.semaphore("spk_sem"))
        m16d_sem = st.enter_context(nc.semaphore("m16d_sem"))
        m7_sem = st.enter_context(nc.semaphore("m7_sem"))
        mo_sem = st.enter_context(nc.semaphore("mo_sem"))
        so_sem = st.enter_context(nc.semaphore("so_sem"))
        block = st.enter_context(nc.Block())

        xslot_sems = [xs0_sem, xs1_sem, xs2_sem]

        def xb_r(k):
            return xb_all[:, (k % 3) * BW : (k % 3 + 1) * BW]

        def wsl(k, tl):
            r = k % 2
            return w_all[:, (r * TBLK + tl) * NL : (r * TBLK + tl + 1) * NL]

        def wblk(k):
            r = k % 2
            return w_all[:, r * BW : (r + 1) * BW]

        def psl(k, tl):
            r = k % 2
            return p_all[:, (r * TBLK + tl) * NL : (r * TBLK + tl + 1) * NL]

        def pblk(k):
            r = k % 2
            return p_all[:, r * BW : (r + 1) * BW]

        def uhsl(t):
            r = t % 2
            return uh_t[:, r * NL : (r + 1) * NL]

        def m16sl(j):
            r = j % 2
            return m16_all[:, r * BW : (r + 1) * BW]

        @block.sync
        def _(sync):
            # consts first (tiny, unblock ACT replication), then x0 (b=0,1;
            # sigth + the other x0 half are issued from the ACT queue)
            for src, dst in ((cm_d, cm_t), (thcm_d, thcm16_t)):
                sync.dma_start(out=dst[:, :], in_=src[:, :]).then_inc(c_sem, 16)
            # x block 0 in t-halves spread over 3 queues (scalar: b2/b3 h0;
            # gpsimd: b2/b3 h1); first half unblocks the chain early
            for b in (0, 1):
                sync.dma_start(
                    out=bv_h(xb_r(0), b, 0), in_=x_src_h(b, 0, 0)
                ).then_inc(xh0_sem, 16)
            for b in (0, 1):
                sync.dma_start(
                    out=bv_h(xb_r(0), b, 1), in_=x_src_h(b, 0, 1)
                ).then_inc(xslot_sems[0], 16)
            # x1/x2 wait for x0 to land so they don't steal its bandwidth
            sync.wait_ge(xslot_sems[0], 64)
            for k in (1, 2):
                for b in range(BL):
                    sync.dma_start(out=bv(xb_r(k), b), in_=x_src(b, k)).then_inc(
                        xslot_sems[k % 3], 16
                    )
            # mid-loop x input DMAs are issued from the ACT queue (the other
            # HWDGE ring) so they never serialize behind the output waits here.
            # mems before spikes: m16d clears mid-block, spk only at block end.
            for k in range(NBLK):
                if k >= 1:
                    sync.wait_ge(m16d_sem, k)
                    for b in range(BL):
                        sync.dma_start(
                            out=out_dst(mems16, b, k - 1),
                            in_=bv(m16sl(k - 1), b),
                        ).then_inc(mo_sem, 16)
                if k < NBLK - 1:
                    sync.wait_ge(spk_sem, k + 1)
                    for b in range(BL):
                        sync.dma_start(
                            out=out_dst(spikes8, b, k), in_=bv(s8_t, b)
                        ).then_inc(so_sem, 16)
            # tail: halved outputs of the last block, interleaved by readiness
            # (mems-half h at m16d >= 8+h, spikes-half h at spk >= 8+h)
            kl = NBLK - 1
            for h in (0, 1):
                sync.wait_ge(m7_sem, h + 1)
                for b in range(BL):
                    sync.dma_start(
                        out=out_dst_h(mems16, b, kl, h),
                        in_=bv_h(m16sl(kl), b, h),
                    ).then_inc(mo_sem, 16)
                sync.wait_ge(spk_sem, NBLK + h)
                for b in range(BL):
                    sync.dma_start(
                        out=out_dst_h(spikes8, b, kl, h), in_=bv_h(s8_t, b, h)
                    ).then_inc(so_sem, 16)
            sync.wait_ge(so_sem, 64 * (NBLK - 1) + 128)
            sync.wait_ge(mo_sem, 64 * (NBLK - 1) + 128)

        @block.vector
        def _(vector):
            HBW = BW // 2
            vector.wait_ge(c_sem, 32)  # cm_t + thcm16_t loaded
            for k in range(NBLK):
                if k == 0:
                    # first half of xs only; chain starts 1 half-DMA earlier
                    vector.wait_ge(rep_sem, TBLK // 2)
                    vector.wait_ge(xh0_sem, 64)
                    nc.vector.tensor_tensor(
                        out=xs_t[:, 0:HBW],
                        in0=xb_r(0)[:, 0:HBW],
                        in1=sigthb[:, 0:HBW],
                        op=ALU.mult,
                    )
                else:
                    vector.wait_ge(xslot_sems[k % 3], 64 * (k // 3 + 1))
                    if k >= 2:
                        # p ring slot k%2: ACT cast of block k-2 must be done
                        vector.wait_ge(p16_sem, k - 1)
                    # xs for block k
                    nc.vector.tensor_tensor(
                        out=xs_t[:, :],
                        in0=xb_r(k),
                        in1=sigthb[:, :],
                        op=ALU.mult,
                    ).then_inc(xsd_sem, 1)
                if k >= 1:
                    # deferred add: uh_{8k} = p_{8k-1} + xs_{8k}
                    nc.vector.tensor_tensor(
                        out=uhsl(8 * k),
                        in0=psl(k - 1, TBLK - 1),
                        in1=xs_t[:, 0:NL],
                        op=ALU.add,
                    )
                if k >= 2:
                    vector.wait_ge(spk_sem, k - 1)  # w ring WAR vs ACT exp
                for tl in range(TBLK):
                    t = k * TBLK + tl
                    if k == 0 and tl == 3:
                        # second xs half (t3's uh-add reads xs slice 4)
                        vector.wait_ge(rep_sem, TBLK)
                        vector.wait_ge(xslot_sems[0], 64)
                        nc.vector.tensor_tensor(
                            out=xs_t[:, HBW:BW],
                            in0=xb_r(0)[:, HBW:BW],
                            in1=sigthb[:, HBW:BW],
                            op=ALU.mult,
                        ).then_inc(xsd_sem, 1)
                    if k >= 1 and tl == 4:
                        # mems16 for block k-1, placed mid-chain so the ACT
                        # cast (done ~7us after chain k-1) is never waited on
                        vector.wait_ge(p16_sem, k)
                        if k >= 3:
                            vector.wait_ge(mo_sem, 64 * (k - 2))  # m16 WAR
                        nc.vector.tensor_tensor(
                            out=m16sl(k - 1),
                            in0=p16_t[:, :],
                            in1=thcmb16[:, :],
                            op=ALU.mult,
                        ).then_inc(m16d_sem, 1)
                    uh = xs_t[:, 0:NL] if t == 0 else uhsl(t)
                    ins_w = nc.vector.scalar_tensor_tensor(
                        out=wsl(k, tl),
                        in0=uh,
                        scalar=1.0,
                        in1=cm_t[:, :],
                        op0=ALU.is_lt,
                        op1=ALU.mult,
                    )
                    if tl == TBLK - 1 or (
                        k == NBLK - 1 and tl == TBLK // 2 - 1
                    ):
                        ins_w.then_inc(w_sem, 1)
                    ins_p = nc.vector.tensor_tensor(
                        out=psl(k, tl), in0=uh, in1=wsl(k, tl), op=ALU.mult
                    )
                    if tl == TBLK - 1:
                        ins_p.then_inc(pb_sem, 1)
                    if k == NBLK - 1:
                        # last block: mems per-t straight from fp32 p (mixed
                        # dtype TT) so the final mems DMA starts immediately
                        if tl == 0:
                            vector.wait_ge(mo_sem, 64 * (NBLK - 2))
                        ins_m = nc.vector.tensor_tensor(
                            out=m16sl(k)[:, tl * NL : (tl + 1) * NL],
                            in0=psl(k, tl),
                            in1=thcmb16[:, tl * NL : (tl + 1) * NL],
                            op=ALU.mult,
                        )
                        if tl in (TBLK // 2 - 1, TBLK - 1):
                            ins_m.then_inc(m7_sem, 1)
                    if tl < TBLK - 1:
                        nc.vector.tensor_tensor(
                            out=uhsl(t + 1),
                            in0=psl(k, tl),
                            in1=xs_t[:, (tl + 1) * NL : (tl + 2) * NL],
                            op=ALU.add,
                        )

        @block.gpsimd
        def _(gp):
            for b in (2, 3):
                nc.gpsimd.dma_start(
                    out=bv_h(xb_r(0), b, 1), in_=x_src_h(b, 0, 1)
                ).then_inc(xslot_sems[0], 16)

        @block.scalar
        def _(scalar):
            # sigth + other half of x block 0 (parallel with the sync queue)
            nc.scalar.dma_start(out=sigth_t[:, :], in_=sigth_d[:, :]).then_inc(
                sg_sem, 16
            )
            for b in (2, 3):
                nc.scalar.dma_start(
                    out=bv_h(xb_r(0), b, 0), in_=x_src_h(b, 0, 0)
                ).then_inc(xh0_sem, 16)
            scalar.wait_ge(sg_sem, 16)  # sigth_t loaded
            for tl in range(TBLK):
                nc.scalar.copy(
                    out=sigthb[:, tl * NL : (tl + 1) * NL], in_=sigth_t[:, :]
                ).then_inc(rep_sem, 1)
            scalar.wait_ge(c_sem, 32)
            for tl in range(TBLK):
                nc.scalar.copy(
                    out=thcmb16[:, tl * NL : (tl + 1) * NL], in_=thcm16_t[:, :]
                ).then_inc(rep_sem, 1)
            for k in range(NBLK):
                # spikes block k first (gates DVE's w ring + sync's s8-out):
                # w==0 iff spike; exp(-1e30*w) = 1/0 exactly
                if k < NBLK - 1:
                    scalar.wait_ge(w_sem, k + 1)
                    if k >= 1:
                        scalar.wait_ge(so_sem, 64 * k)  # s8 WAR
                    nc.scalar.activation(
                        s8_t[:, :], wblk(k), AF.Exp, scale=-1e30
                    ).then_inc(spk_sem, 1)
                else:
                    # last block: spikes in halves so output DMA starts early
                    scalar.wait_ge(so_sem, 64 * k)
                    half = BW // 2
                    for h in (0, 1):
                        scalar.wait_ge(w_sem, k + 1 + h)
                        nc.scalar.activation(
                            s8_t[:, h * half : (h + 1) * half],
                            wblk(k)[:, h * half : (h + 1) * half],
                            AF.Exp,
                            scale=-1e30,
                        ).then_inc(spk_sem, 1)
                # issue x input DMA for block k+3 (other HWDGE ring; gated
                # only on the xs-op that frees the ring slot)
                kf = k + 3
                if kf < NBLK:
                    scalar.wait_ge(xsd_sem, k + 1)
                    for b in range(BL):
                        nc.scalar.dma_start(
                            out=bv(xb_r(kf), b), in_=x_src(b, kf)
                        ).then_inc(xslot_sems[kf % 3], 16)
                # p block k -> fp16 (for the all-16-bit mems mult); skipped
                # for the last block (its mems come straight from fp32 p)
                if k < NBLK - 1:
                    scalar.wait_ge(pb_sem, k + 1)
                    if k >= 1:
                        scalar.wait_ge(m16d_sem, k)  # p16 used by m16-op k-1
                    nc.scalar.copy(out=p16_t[:, :], in_=pblk(k)).then_inc(
                        p16_sem, 1
                    )

    return nc


def _get_nc() -> bass.Bass:
    if "nc" not in _CACHE:
        _CACHE["nc"] = _build_nc()
    return _CACHE["nc"]


def kernel(x, thresh, tau_x, _trace: bool = False, _tmpdir: str | None = None):
    x = np.ascontiguousarray(np.asarray(x, dtype=np.float32))
    thresh = np.ascontiguousarray(np.asarray(thresh, dtype=np.float32))
    tau_x = np.ascontiguousarray(np.asarray(tau_x, dtype=np.float32))
    assert x.shape == (B, T, N)

    # O(N) host-side constants; all O(B*T*N) math happens on-device.
    sig = (1.0 / (1.0 + np.exp(-tau_x.astype(np.float64)))).astype(np.float32)
    cm = (np.float32(1.0) - sig).astype(np.float32)
    sigth = (sig / thresh).astype(np.float32)
    thcm16 = (thresh / cm).astype(np.float16)
    # pretile to [128, 512]: partition p = b_local*32 + s holds neuron
    # chunk s; replicate the [32, 512] view across the 4 b-groups
    cm_tl = np.ascontiguousarray(np.tile(cm.reshape(SUB, NL), (BL, 1)))
    sigth_tl = np.ascontiguousarray(np.tile(sigth.reshape(SUB, NL), (BL, 1)))
    thcm16_tl = np.ascontiguousarray(np.tile(thcm16.reshape(SUB, NL), (BL, 1)))

    nc = _get_nc()
    in_maps = [
        {
            "x": x[i * BL : (i + 1) * BL],
            "cm": cm_tl,
            "sigth": sigth_tl,
            "thcm16": thcm16_tl,
        }
        for i in range(NCORES)
    ]
    res = run_bass_kernel_spmd(
        nc, in_maps, core_ids=list(range(NCORES)), trace=_trace, tmpdir=_tmpdir
    )
    spikes = np.concatenate(
        [np.asarray(r["spikes8"]).astype(np.float32) for r in res.results],
        axis=0,
    )
    mems = np.concatenate(
        [np.asarray(r["mems16"]).astype(np.float32) for r in res.results],
        axis=0,
    )
    if _trace:
        _CACHE["last_results"] = res
    return spikes, mems
